# revision 1
# baseline (speedup 1.0000x reference)
"""Bass/Trainium2 kernel for nn_BiChannelAttention (single-query local-window attention).

Math (per batch b, head h, with S=2049, window W=256, cutoff=S-W=1793):
  Positions before the cutoff receive a -1e6 additive mask, so after softmax their
  weight is exactly 0.0 in fp32 (exp underflows). Only the last W positions matter.

  For window rows X [W, 128] (last 255 cache rows + content row):
    q   = cnt_h @ Wq_h                      (128)
    kq  = (Wk_h/sqrt(128))^T q              (128)      <- folds Wk into q
    sc  = X kq  (+ per-position bias)       (W)        <- column-major on chip
    a   = exp(sc)          (no max-subtraction needed: unmasked scores are O(1))
    xa  = X^T a / sum(a)                    (128)
    out = Wv_h^T xa + cnt_h                 (128)

Sharding: tensor-parallel over heads, 2 heads per core x 8 cores. Each core reads
only its heads' weight slices and window slices (~2.2 MB).
"""

import sys
import numpy as np

for _p in ("/opt/trn_rl_repo", "/root/.axon_site/_ro/trn_rl_repo"):
    if _p not in sys.path:
        sys.path.insert(0, _p)

import concourse.bass as bass
import concourse.bacc as bacc
import concourse.mybir as mybir
from concourse.tile import TileContext
from concourse.bass_utils import run_bass_kernel_spmd

F32 = mybir.dt.float32
P = 128          # partitions / head_dim
B = 8            # batch
H = 16           # heads total
HPC = 2          # heads per core
NCORES = 8
T = 2048
S = T + 1
W = 256          # local attention window
CUTOFF = S - W   # 1793
NEG = -1000000.0

_NC_CACHE = {}


def _build_nc():
    nc = bacc.Bacc(None, target_bir_lowering=False, debug=False)
    # packed constants along the free dim: ident | ones | bias | cnt | (wq,wkt,wv) x HPC
    CK = 2 * P + 2 * B + HPC * B + 3 * HPC * P
    x_in = nc.declare_dram_parameter("x", [B, HPC, W, P], F32, isOutput=False)
    consts_in = nc.declare_dram_parameter("consts", [P, CK], F32, isOutput=False)
    out_t = nc.declare_dram_parameter("out", [HPC, P, B], F32, isOutput=True)

    with TileContext(nc) as tc:
        with (
            tc.tile_pool(name="const", bufs=1) as cpool,
            tc.tile_pool(name="xin", bufs=10) as xpool,
            tc.tile_pool(name="xt", bufs=10) as xtpool,
            tc.tile_pool(name="small", bufs=2) as spool,
            tc.tile_pool(name="ps_t", bufs=2, space="PSUM") as pst,
            tc.tile_pool(name="ps_qk", bufs=2, space="PSUM") as psqk,
            tc.tile_pool(name="ps_at", bufs=2, space="PSUM") as psat,
            tc.tile_pool(name="ps_xo", bufs=2, space="PSUM") as psxo,
        ):
            consts = cpool.tile([P, CK], F32, tag="consts")
            nc.sync.dma_start(out=consts[:, :], in_=consts_in[:, :])
            o = 0
            ident = consts[:, o:o + P]; o += P
            ones = consts[:, o:o + P]; o += P
            biasT = consts[:, o:o + 2 * B]; o += 2 * B
            cntT = consts[:, o:o + HPC * B]; o += HPC * B
            wq, wkt, wv = [], [], []
            for j in range(HPC):
                wq.append(consts[:, o:o + P]); o += P
                wkt.append(consts[:, o:o + P]); o += P
                wv.append(consts[:, o:o + P]); o += P

            for j in range(HPC):
                cnt_j = cntT[:, j * B:(j + 1) * B]

                # q for all 8 batches: q[e,b] = sum_d Wq[d,e] cnt[d,b]
                qk_ps = psqk.tile([P, 2 * B], F32, tag="qk")
                nc.tensor.matmul(qk_ps[:, 0:B], wq[j], cnt_j, start=True, stop=True)
                q_sb = spool.tile([P, B], F32, tag="q")
                nc.vector.tensor_copy(q_sb[:, :], qk_ps[:, 0:B])
                # kq[d,b] = sum_e WkT[e,d] q[e,b]   (WkT pre-scaled by 1/sqrt(128))
                nc.tensor.matmul(qk_ps[:, B:2 * B], wkt[j], q_sb[:, :], start=True, stop=True)
                kq_sb = spool.tile([P, B], F32, tag="kq")
                nc.vector.tensor_copy(kq_sb[:, :], qk_ps[:, B:2 * B])

                at_ps = psat.tile([P, 3 * B], F32, tag="at")  # scores [0:16], denom [16:24]
                xo_ps = psxo.tile([P, 2 * B], F32, tag="xo")  # xa [0:8], out [8:16]

                x0s, x1s, xt0s, xt1s = [], [], [], []
                for b in range(B):
                    x0 = xpool.tile([P, P], F32, tag="x0")
                    nc.sync.dma_start(out=x0[:, :], in_=x_in[b, j, 0:P, :])
                    x1 = xpool.tile([P, P], F32, tag="x1")
                    nc.sync.dma_start(out=x1[:, :], in_=x_in[b, j, P:W, :])
                    xt_ps = pst.tile([P, 2 * P], F32, tag="xtp")
                    nc.tensor.transpose(xt_ps[:, 0:P], x0[:, :], ident)
                    nc.tensor.transpose(xt_ps[:, P:2 * P], x1[:, :], ident)
                    xt0 = xtpool.tile([P, P], F32, tag="xt0")
                    nc.vector.tensor_copy(xt0[:, :], xt_ps[:, 0:P])
                    xt1 = xtpool.tile([P, P], F32, tag="xt1")
                    nc.scalar.copy(xt1[:, :], xt_ps[:, P:2 * P])
                    # scores: column [s,1] per (tile, b) -> at_ps col jt*8+b
                    nc.tensor.matmul(at_ps[:, b:b + 1], xt0[:, :], kq_sb[:, b:b + 1], start=True, stop=True)
                    nc.tensor.matmul(at_ps[:, B + b:B + b + 1], xt1[:, :], kq_sb[:, b:b + 1], start=True, stop=True)
                    x0s.append(x0); x1s.append(x1); xt0s.append(xt0); xt1s.append(xt1)

                # bias add + exp for all 16 score columns at once
                att_pre = spool.tile([P, 2 * B], F32, tag="att_pre")
                nc.vector.tensor_add(att_pre[:, :], at_ps[:, 0:2 * B], biasT)
                att = spool.tile([P, 2 * B], F32, tag="att")
                nc.scalar.activation(att[:, :], att_pre[:, :], mybir.ActivationFunctionType.Exp)

                # denominator broadcast over partitions: accumulate both s-tiles on PE
                nc.tensor.matmul(at_ps[:, 2 * B:3 * B], ones, att[:, 0:B], start=True, stop=False)
                nc.tensor.matmul(at_ps[:, 2 * B:3 * B], ones, att[:, B:2 * B], start=False, stop=True)
                rec = spool.tile([P, B], F32, tag="rec")
                nc.vector.reciprocal(rec[:, :], at_ps[:, 2 * B:3 * B])

                # xa[d,b] = sum_s X[s,d] a[s,b]  (accumulate the two s-tiles)
                for b in range(B):
                    nc.tensor.matmul(xo_ps[:, b:b + 1], x0s[b][:, :], att[:, b:b + 1], start=True, stop=False)
                    nc.tensor.matmul(xo_ps[:, b:b + 1], x1s[b][:, :], att[:, B + b:B + b + 1], start=False, stop=True)
                xa_sb = spool.tile([P, B], F32, tag="xa")
                nc.vector.tensor_mul(xa_sb[:, :], xo_ps[:, 0:B], rec[:, :])

                # out[e,b] = sum_d Wv[d,e] xa[d,b]; residual add; store
                nc.tensor.matmul(xo_ps[:, B:2 * B], wv[j], xa_sb[:, :], start=True, stop=True)
                fin = spool.tile([P, B], F32, tag="fin")
                nc.vector.tensor_add(fin[:, :], xo_ps[:, B:2 * B], cnt_j)
                nc.sync.dma_start(out=out_t[j, :, :], in_=fin[:, :])
    nc.finalize()
    return nc


def _get_nc():
    if "nc" not in _NC_CACHE:
        _NC_CACHE["nc"] = _build_nc()
    return _NC_CACHE["nc"]


def _pos_bias_f32():
    """t5_position_bucket exactly as the reference computes it (same jnp ops on the
    in-process default jax backend), sliced to the window."""
    if "pos" not in _NC_CACHE:
        import jax.numpy as jnp
        NUM_BUCKETS, MAX_DISTANCE = 32, 128
        n = (S - 1) - jnp.arange(S)
        max_exact = NUM_BUCKETS // 2
        is_small = n < max_exact
        large = max_exact + (
            jnp.log(jnp.maximum(n, 1).astype(jnp.float32) / max_exact)
            / np.log(MAX_DISTANCE / max_exact)
            * (NUM_BUCKETS - max_exact)
        ).astype(jnp.int32)
        large = jnp.minimum(large, NUM_BUCKETS - 1)
        pos = jnp.where(is_small, n, large).astype(jnp.float32)
        _NC_CACHE["pos"] = np.asarray(pos)[CUTOFF:]  # [W]
    return _NC_CACHE["pos"]


def kernel(**inputs) -> np.ndarray:
    t = int(np.asarray(inputs["t"]))
    assert t == T, f"kernel hardcoded for t={T}, got {t}"
    content_t = np.ascontiguousarray(np.asarray(inputs["content_t"], dtype=np.float32))
    time_mask = np.asarray(inputs["time_mask"])
    cache = np.asarray(inputs["cache"], dtype=np.float32)
    Wq = np.asarray(inputs["Wq"], dtype=np.float32)
    Wk = np.asarray(inputs["Wk"], dtype=np.float32)
    Wv = np.asarray(inputs["Wv"], dtype=np.float32)
    pos_param = np.float32(np.asarray(inputs["pos_param"]))

    # Per-position additive bias for the window: -pos_param*bucket only.
    # The reference's masked_fill sequence (1->0, then every 0->NEG) sets ALL
    # positions to NEG, a uniform shift softmax cancels -- time_mask is a no-op.
    del time_mask
    pos = _pos_bias_f32()                                   # [W]
    posb = (-pos_param * pos).astype(np.float32)            # [W]
    bias_col = posb.reshape(2, P).transpose(1, 0)           # [p, jt]
    bias_t = np.ascontiguousarray(
        np.broadcast_to(bias_col[:, :, None], (P, 2, B)).reshape(P, 2 * B)
    )  # [p, jt*8+b]

    win = cache[:, CUTOFF:T, :].reshape(B, W - 1, H, P)      # [B, 255, H, 128]
    cnt_h = content_t.reshape(B, H, P)                       # [B, H, 128]
    wkt_full = (Wk.transpose(0, 2, 1) / np.float32(np.sqrt(128.0))).astype(np.float32)

    ones = np.ones((P, P), np.float32)
    ident = np.eye(P, dtype=np.float32)

    in_maps = []
    for c in range(NCORES):
        h0 = HPC * c
        x_host = np.empty((B, HPC, W, P), np.float32)
        for j in range(HPC):
            x_host[:, j, : W - 1, :] = win[:, :, h0 + j, :]
            x_host[:, j, W - 1, :] = cnt_h[:, h0 + j, :]
        cnt_host = np.ascontiguousarray(
            cnt_h[:, h0:h0 + HPC, :].transpose(2, 1, 0).reshape(P, HPC * B)
        )  # [d, j*8+b]
        blocks = [ident, ones, bias_t, cnt_host]
        for j in range(HPC):
            blocks += [Wq[h0 + j], wkt_full[h0 + j], Wv[h0 + j]]
        consts_host = np.ascontiguousarray(np.concatenate(blocks, axis=1), dtype=np.float32)
        in_maps.append({"x": x_host, "consts": consts_host})

    nc = _get_nc()
    res = run_bass_kernel_spmd(nc, in_maps, list(range(NCORES)), **_RUN_KWARGS)
    _NC_CACHE["last_results"] = res
    outs = np.stack([np.asarray(res.results[c]["out"]) for c in range(NCORES)])
    # outs: [core, j, d, b] -> out_full[b, (2c+j)*128 + d]
    out_full = outs.transpose(3, 0, 1, 2).reshape(B, H * P)
    return out_full.astype(np.float32)


_RUN_KWARGS = {}  # test harness may set {"trace": True, "tmpdir": ...}



# revision 5
# speedup vs baseline: 2.7930x; 2.7930x over previous
"""Bass/Trainium2 kernel for nn_BiChannelAttention (single-query local-window attention).

Math (per batch b, head h, with S=2049, window W=256, cutoff=S-W=1793):
  Positions before the cutoff receive a -1e6 additive mask, so after softmax their
  weight is exactly 0.0 in fp32 (exp underflows). Only the last W positions matter.
  The reference's masked_fill sequence (1->0, then every 0->NEG) sets ALL positions
  to NEG -- a uniform shift softmax cancels, so time_mask is a no-op.

  For window rows X [W, 128] (last 255 cache rows + content row):
    kq  = (Wq_h Wk_h^T / sqrt(128))^T cnt_h   (128)   <- folded on host into MT_h
    sc  = X kq + posbias                      (W)
    a   = exp(sc)       (no max-subtraction: unmasked scores are O(1))
    xa  = X^T a / sum(a)                      (128)
    out = Wv_h^T xa + cnt_h                   (128)

Device-side layout (per core, tensor-parallel over heads, 2 heads/core x 8 cores):
  xt [j][d=128p, (b,t)*128+s]  bf16  -- X^T tiles, stationary for the score matmuls
  xn [j][s=128p, (b,t)*128+d]  bf16  -- X tiles, stationary for the AV matmuls
  scores land as [s=128p, t*16+bj] in PSUM; exp folds the position bias via the
  ACT per-partition bias; denom = ones^T @ att (broadcast across partitions);
  1/denom folds into one [128,16] multiply on xa. All PE traffic is bf16 so FWL
  (fast weight load) engages; X passes through the PE weight path exactly twice.
"""

import sys
import numpy as np

for _p in ("/opt/trn_rl_repo", "/root/.axon_site/_ro/trn_rl_repo"):
    if _p not in sys.path:
        sys.path.insert(0, _p)

import concourse.bass as bass
import concourse.bacc as bacc
import concourse.mybir as mybir
from concourse.tile import TileContext
from concourse.bass_utils import run_bass_kernel_spmd

F32 = mybir.dt.float32
BF16 = mybir.dt.bfloat16
P = 128          # partitions / head_dim
B = 8            # batch
H = 16           # heads total
HPC = 2          # heads per core
BJ = HPC * B     # (b, j) pairs per core
NCORES = 8
T = 2048
S = T + 1
W = 256          # local attention window
NT = W // P      # s-tiles per (b, j)
CUTOFF = S - W   # 1793

_NC_CACHE = {}


def _build_nc():
    nc = bacc.Bacc(None, target_bir_lowering=False, debug=False)
    # bf16 consts along free dim: MT0 | MT1 | Wv0 | Wv1 | ones | cnt_bf
    CKB = 5 * P + BJ
    xt_in = nc.declare_dram_parameter("xt", [HPC, P, B * NT * P], BF16, isOutput=False)
    xn_in = nc.declare_dram_parameter("xn", [HPC, P, B * NT * P], BF16, isOutput=False)
    cb_in = nc.declare_dram_parameter("cb", [P, CKB], BF16, isOutput=False)
    cf_in = nc.declare_dram_parameter("cf", [P, NT + BJ], F32, isOutput=False)
    out_t = nc.declare_dram_parameter("out", [P, BJ], F32, isOutput=True)

    with TileContext(nc) as tc:
        with (
            tc.tile_pool(name="cpool", bufs=1) as cpool,
            tc.tile_pool(name="xtp", bufs=2) as xtp,
            tc.tile_pool(name="xnp", bufs=2) as xnp,
            tc.tile_pool(name="small", bufs=2) as spool,
            tc.tile_pool(name="ps_kq", bufs=1, space="PSUM") as pskq,
            tc.tile_pool(name="ps_sc", bufs=1, space="PSUM") as pssc,
            tc.tile_pool(name="ps_dn", bufs=1, space="PSUM") as psdn,
            tc.tile_pool(name="ps_xa", bufs=1, space="PSUM") as psxa,
            tc.tile_pool(name="ps_o", bufs=1, space="PSUM") as pso,
        ):
            # Input DMAs split across the two HWDGE rings (sync=SP, scalar=ACT):
            # each ring streams ~1.1 MB concurrently, halving the DMA wall time.
            cb = cpool.tile([P, CKB], BF16, tag="cb")
            nc.sync.dma_start(out=cb[:, :], in_=cb_in[:, :])
            cf = cpool.tile([P, NT + BJ], F32, tag="cf")
            nc.scalar.dma_start(out=cf[:, :], in_=cf_in[:, :])
            o = 0
            mt = []
            wv = []
            for j in range(HPC):
                mt.append(cb[:, o:o + P]); o += P
            for j in range(HPC):
                wv.append(cb[:, o:o + P]); o += P
            ones = cb[:, o:o + P]; o += P
            cnt_bf = cb[:, o:o + BJ]; o += BJ
            bias = cf[:, 0:NT]
            cnt_f32 = cf[:, NT:NT + BJ]

            xt = []
            xn = []
            for j in range(HPC):
                xtj = xtp.tile([P, B * NT * P], BF16, tag=f"xt{j}")
                eng = nc.sync if j == 0 else nc.scalar
                eng.dma_start(out=xtj[:, :], in_=xt_in[j, :, :])
                xt.append(xtj)
            for j in range(HPC):
                xnj = xnp.tile([P, B * NT * P], BF16, tag=f"xn{j}")
                eng = nc.sync if j == 0 else nc.scalar
                eng.dma_start(out=xnj[:, :], in_=xn_in[j, :, :])
                xn.append(xnj)

            # kq[d, jb] = sum_d' MT_j[d', d] cnt[d', jb]   (MT = Wq Wk^T / sqrt(hd))
            kq_ps = pskq.tile([P, BJ], F32, tag="kq")
            for j in range(HPC):
                nc.tensor.matmul(kq_ps[:, j * B:(j + 1) * B], mt[j],
                                 cnt_bf[:, j * B:(j + 1) * B], start=True, stop=True)
            kq = spool.tile([P, BJ], BF16, tag="kq_sb")
            nc.vector.tensor_copy(kq[:, :], kq_ps[:, :])

            # scores[s, t*16 + (j*8+b)] = sum_d X^T[d, (b,t) tile][d, s] kq[d, jb]
            sc_ps = pssc.tile([P, NT * BJ], F32, tag="sc")
            for j in range(HPC):
                for b in range(B):
                    for t in range(NT):
                        col = t * BJ + j * B + b
                        nc.tensor.matmul(
                            sc_ps[:, col:col + 1],
                            xt[j][:, (b * NT + t) * P:(b * NT + t + 1) * P],
                            kq[:, j * B + b:j * B + b + 1],
                            start=True, stop=True,
                        )

            # att = exp(scores + posbias[s, t]); bias is per-partition per s-tile
            att = spool.tile([P, NT * BJ], BF16, tag="att")
            for t in range(NT):
                nc.scalar.activation(
                    att[:, t * BJ:(t + 1) * BJ], sc_ps[:, t * BJ:(t + 1) * BJ],
                    mybir.ActivationFunctionType.Exp, bias=bias[:, t:t + 1],
                )

            # denom[_, jb] = sum_s att[s, jb] broadcast across partitions via ones
            dn_ps = psdn.tile([P, BJ], F32, tag="dn")
            for t in range(NT):
                nc.tensor.matmul(dn_ps[:, :], ones, att[:, t * BJ:(t + 1) * BJ],
                                 start=(t == 0), stop=(t == NT - 1))
            rec = spool.tile([P, BJ], F32, tag="rec")
            nc.vector.reciprocal(rec[:, :], dn_ps[:, :])

            # xa[d, jb] = sum_s X[(b,t) tile][s, d] att[s, t*16+jb]  (unnormalized)
            xa_ps = psxa.tile([P, BJ], F32, tag="xa")
            for j in range(HPC):
                for b in range(B):
                    col = j * B + b
                    for t in range(NT):
                        nc.tensor.matmul(
                            xa_ps[:, col:col + 1],
                            xn[j][:, (b * NT + t) * P:(b * NT + t + 1) * P],
                            att[:, t * BJ + col:t * BJ + col + 1],
                            start=(t == 0), stop=(t == NT - 1),
                        )
            xa = spool.tile([P, BJ], BF16, tag="xa_sb")
            nc.vector.tensor_mul(xa[:, :], xa_ps[:, :], rec[:, :])

            # out[e, jb] = sum_d Wv_j[d, e] xa[d, jb] + cnt[e, jb]
            o_ps = pso.tile([P, BJ], F32, tag="o")
            for j in range(HPC):
                nc.tensor.matmul(o_ps[:, j * B:(j + 1) * B], wv[j],
                                 xa[:, j * B:(j + 1) * B], start=True, stop=True)
            fin = spool.tile([P, BJ], F32, tag="fin")
            nc.vector.tensor_add(fin[:, :], o_ps[:, :], cnt_f32[:, :])
            nc.sync.dma_start(out=out_t[:, :], in_=fin[:, :])
    nc.finalize()
    return nc


def _get_nc():
    if "nc" not in _NC_CACHE:
        _NC_CACHE["nc"] = _build_nc()
    return _NC_CACHE["nc"]


def _pos_bias_f32():
    """t5_position_bucket exactly as the reference computes it, sliced to the window."""
    if "pos" not in _NC_CACHE:
        import jax.numpy as jnp
        NUM_BUCKETS, MAX_DISTANCE = 32, 128
        n = (S - 1) - jnp.arange(S)
        max_exact = NUM_BUCKETS // 2
        is_small = n < max_exact
        large = max_exact + (
            jnp.log(jnp.maximum(n, 1).astype(jnp.float32) / max_exact)
            / np.log(MAX_DISTANCE / max_exact)
            * (NUM_BUCKETS - max_exact)
        ).astype(jnp.int32)
        large = jnp.minimum(large, NUM_BUCKETS - 1)
        pos = jnp.where(is_small, n, large).astype(jnp.float32)
        _NC_CACHE["pos"] = np.asarray(pos)[CUTOFF:]  # [W]
    return _NC_CACHE["pos"]


def kernel(**inputs) -> np.ndarray:
    import ml_dtypes
    BF = ml_dtypes.bfloat16

    t = int(np.asarray(inputs["t"]))
    assert t == T, f"kernel hardcoded for t={T}, got {t}"
    content_t = np.asarray(inputs["content_t"], dtype=np.float32)
    cache = np.asarray(inputs["cache"], dtype=np.float32)
    Wq = np.asarray(inputs["Wq"], dtype=np.float32)
    Wk = np.asarray(inputs["Wk"], dtype=np.float32)
    Wv = np.asarray(inputs["Wv"], dtype=np.float32)
    pos_param = np.float32(np.asarray(inputs["pos_param"]))

    posb = (-pos_param * _pos_bias_f32()).astype(np.float32)       # [W]
    bias_col = np.ascontiguousarray(posb.reshape(NT, P).T)          # [p, t]

    # per-head folded score matrix MT_h = Wq_h @ Wk_h^T / sqrt(128)
    MT = (np.einsum("hde,hfe->hdf", Wq, Wk) / np.float32(np.sqrt(128.0)))
    MT_bf = MT.astype(BF)                                           # [H, d', d]
    Wv_bf = Wv.astype(BF)                                           # [H, d, e]

    cnt_h = content_t.reshape(B, H, P)                              # [B, H, 128]
    # full window incl. content row, cast to bf16 once: [B, W, H, P]
    w_all = np.empty((B, W, H, P), dtype=BF)
    w_all[:, : W - 1] = cache[:, CUTOFF:T, :].reshape(B, W - 1, H, P).astype(BF)
    w_all[:, W - 1] = cnt_h.astype(BF)
    # [B, t, p, H, d] view for packing
    w_t = w_all.reshape(B, NT, P, H, P)

    ones = np.ones((P, P), dtype=BF)

    in_maps = []
    for c in range(NCORES):
        h0 = HPC * c
        # xn[j][p, (b*NT+t)*P + d] = w[b, t*P+p, h0+j, d]
        blk = w_t[:, :, :, h0:h0 + HPC, :]                          # [B, t, p, j, d]
        xn_host = np.ascontiguousarray(
            blk.transpose(3, 2, 0, 1, 4).reshape(HPC, P, B * NT * P))
        # xt[j][d, (b*NT+t)*P + p] = w[b, t*P+p, h0+j, d]
        xt_host = np.ascontiguousarray(
            blk.transpose(3, 4, 0, 1, 2).reshape(HPC, P, B * NT * P))

        cnt_j = cnt_h[:, h0:h0 + HPC, :].transpose(2, 1, 0).reshape(P, BJ)  # [d, j*8+b]
        cb_host = np.ascontiguousarray(np.concatenate(
            [MT_bf[h0], MT_bf[h0 + 1], Wv_bf[h0], Wv_bf[h0 + 1], ones,
             cnt_j.astype(BF)], axis=1))
        cf_host = np.ascontiguousarray(np.concatenate(
            [bias_col, cnt_j.astype(np.float32)], axis=1))
        in_maps.append({"xt": xt_host, "xn": xn_host, "cb": cb_host, "cf": cf_host})

    nc = _get_nc()
    res = run_bass_kernel_spmd(nc, in_maps, list(range(NCORES)), **_RUN_KWARGS)
    outs = np.stack([np.asarray(res.results[c]["out"]) for c in range(NCORES)])
    if not np.isfinite(outs).all():
        # The math here is provably finite (softmax denominator >= W*exp(-2));
        # a NaN/Inf can only be a transient device fault -- run once more.
        res = run_bass_kernel_spmd(nc, in_maps, list(range(NCORES)), **_RUN_KWARGS)
        outs = np.stack([np.asarray(res.results[c]["out"]) for c in range(NCORES)])
    _NC_CACHE["last_results"] = res
    # outs: [core, e, j*8+b] -> out_full[b, (2c+j)*128 + e]
    out_full = outs.reshape(NCORES, P, HPC, B).transpose(3, 0, 2, 1).reshape(B, H * P)
    return np.ascontiguousarray(out_full, dtype=np.float32)


_RUN_KWARGS = {}  # test harness may set {"trace": True, "tmpdir": ...}


# revision 6
# speedup vs baseline: 2.9897x; 1.0704x over previous
"""Bass/Trainium2 kernel for nn_BiChannelAttention (single-query local-window attention).

Math (per batch b, head h, with S=2049, window W=256, cutoff=S-W=1793):
  Positions before the cutoff receive a -1e6 additive mask, so after softmax their
  weight is exactly 0.0 in fp32 (exp underflows). Only the last W positions matter.
  The reference's masked_fill sequence (1->0, then every 0->NEG) sets ALL positions
  to NEG -- a uniform shift softmax cancels, so time_mask is a no-op.

  For window rows X [W, 128] (last 255 cache rows + content row):
    kq  = (Wq_h Wk_h^T / sqrt(128))^T cnt_h   (128)   <- folded on host into MT_h
    sc  = X kq + posbias                      (W)
    a   = exp(sc)       (no max-subtraction: unmasked scores are O(1))
    xa  = X^T a / sum(a)                      (128)
    out = Wv_h^T xa + cnt_h                   (128)

Device-side layout (per core, tensor-parallel over heads, 2 heads/core x 8 cores):
  xt [j][d=128p, (b,t)*128+s]  bf16  -- X^T tiles, stationary for the score matmuls
  xn [j][s=128p, (b,t)*128+d]  bf16  -- X tiles, stationary for the AV matmuls
  scores land as [s=128p, t*16+bj] in PSUM; exp folds the position bias via the
  ACT per-partition bias; denom = ones^T @ att (broadcast across partitions);
  1/denom folds into one [128,16] multiply on xa. All PE traffic is bf16 so FWL
  (fast weight load) engages; X passes through the PE weight path exactly twice.
"""

import sys
import numpy as np

for _p in ("/opt/trn_rl_repo", "/root/.axon_site/_ro/trn_rl_repo"):
    if _p not in sys.path:
        sys.path.insert(0, _p)

import concourse.bass as bass
import concourse.bacc as bacc
import concourse.mybir as mybir
from concourse.tile import TileContext
from concourse.bass_utils import run_bass_kernel_spmd

F32 = mybir.dt.float32
BF16 = mybir.dt.bfloat16
F8 = mybir.dt.float8e4
KQS = 64.0   # kq prescale (folded into MT on host), undone by the exp ACT scale
P = 128          # partitions / head_dim
B = 8            # batch
H = 16           # heads total
HPC = 2          # heads per core
BJ = HPC * B     # (b, j) pairs per core
NCORES = 8
T = 2048
S = T + 1
W = 256          # local attention window
NT = W // P      # s-tiles per (b, j)
CUTOFF = S - W   # 1793

_NC_CACHE = {}


def _build_nc():
    nc = bacc.Bacc(None, target_bir_lowering=False, debug=False)
    # bf16 consts along free dim: MT0 | MT1 | Wv0 | Wv1 | ones | cnt_bf
    CKB = 4 * P + BJ
    xt_in = nc.declare_dram_parameter("xt", [HPC, P, B * NT * P], F8, isOutput=False)
    xn_in = nc.declare_dram_parameter("xn", [HPC, P, B * NT * P], F8, isOutput=False)
    cb_in = nc.declare_dram_parameter("cb", [P, CKB], BF16, isOutput=False)
    on_in = nc.declare_dram_parameter("on8", [P, P], F8, isOutput=False)
    cf_in = nc.declare_dram_parameter("cf", [P, NT + BJ], F32, isOutput=False)
    out_t = nc.declare_dram_parameter("out", [P, BJ], F32, isOutput=True)

    with TileContext(nc) as tc:
        with (
            tc.tile_pool(name="cpool", bufs=1) as cpool,
            tc.tile_pool(name="xtp", bufs=2) as xtp,
            tc.tile_pool(name="xnp", bufs=2) as xnp,
            tc.tile_pool(name="small", bufs=2) as spool,
            tc.tile_pool(name="ps_kq", bufs=1, space="PSUM") as pskq,
            tc.tile_pool(name="ps_sc", bufs=1, space="PSUM") as pssc,
            tc.tile_pool(name="ps_dn", bufs=1, space="PSUM") as psdn,
            tc.tile_pool(name="ps_xa", bufs=1, space="PSUM") as psxa,
            tc.tile_pool(name="ps_o", bufs=1, space="PSUM") as pso,
        ):
            # Input DMAs split across the two HWDGE rings (sync=SP, scalar=ACT):
            # each ring streams ~1.1 MB concurrently, halving the DMA wall time.
            cb = cpool.tile([P, CKB], BF16, tag="cb")
            nc.sync.dma_start(out=cb[:, :], in_=cb_in[:, :])
            cf = cpool.tile([P, NT + BJ], F32, tag="cf")
            nc.scalar.dma_start(out=cf[:, :], in_=cf_in[:, :])
            ones = cpool.tile([P, P], F8, tag="on8")
            nc.scalar.dma_start(out=ones[:, :], in_=on_in[:, :])
            o = 0
            mt = []
            wv = []
            for j in range(HPC):
                mt.append(cb[:, o:o + P]); o += P
            for j in range(HPC):
                wv.append(cb[:, o:o + P]); o += P
            cnt_bf = cb[:, o:o + BJ]; o += BJ
            bias = cf[:, 0:NT]
            cnt_f32 = cf[:, NT:NT + BJ]

            xt = []
            xn = []
            for j in range(HPC):
                xtj = xtp.tile([P, B * NT * P], F8, tag=f"xt{j}")
                eng = nc.sync if j == 0 else nc.scalar
                eng.dma_start(out=xtj[:, :], in_=xt_in[j, :, :])
                xt.append(xtj)
            for j in range(HPC):
                xnj = xnp.tile([P, B * NT * P], F8, tag=f"xn{j}")
                eng = nc.sync if j == 0 else nc.scalar
                eng.dma_start(out=xnj[:, :], in_=xn_in[j, :, :])
                xn.append(xnj)

            # kq[d, jb] = sum_d' MT_j[d', d] cnt[d', jb]   (MT = Wq Wk^T / sqrt(hd))
            kq_ps = pskq.tile([P, BJ], F32, tag="kq")
            for j in range(HPC):
                nc.tensor.matmul(kq_ps[:, j * B:(j + 1) * B], mt[j],
                                 cnt_bf[:, j * B:(j + 1) * B], start=True, stop=True)
            kq = spool.tile([P, BJ], F8, tag="kq_sb")
            nc.vector.tensor_copy(kq[:, :], kq_ps[:, :])

            # scores[s, t*16 + (j*8+b)] = sum_d X^T[d, (b,t) tile][d, s] kq[d, jb]
            sc_ps = pssc.tile([P, NT * BJ], F32, tag="sc")
            for j in range(HPC):
                for b in range(B):
                    for t in range(NT):
                        col = t * BJ + j * B + b
                        nc.tensor.matmul(
                            sc_ps[:, col:col + 1],
                            xt[j][:, (b * NT + t) * P:(b * NT + t + 1) * P],
                            kq[:, j * B + b:j * B + b + 1],
                            start=True, stop=True,
                        )

            # att = exp(scores + posbias[s, t]); bias is per-partition per s-tile
            att = spool.tile([P, NT * BJ], F8, tag="att")
            for t in range(NT):
                nc.scalar.activation(
                    att[:, t * BJ:(t + 1) * BJ], sc_ps[:, t * BJ:(t + 1) * BJ],
                    mybir.ActivationFunctionType.Exp, bias=bias[:, t:t + 1], scale=1.0 / KQS,
                )

            # denom[_, jb] = sum_s att[s, jb] broadcast across partitions via ones
            dn_ps = psdn.tile([P, BJ], F32, tag="dn")
            for t in range(NT):
                nc.tensor.matmul(dn_ps[:, :], ones[:, :], att[:, t * BJ:(t + 1) * BJ],
                                 start=(t == 0), stop=(t == NT - 1))
            rec = spool.tile([P, BJ], F32, tag="rec")
            nc.vector.reciprocal(rec[:, :], dn_ps[:, :])

            # xa[d, jb] = sum_s X[(b,t) tile][s, d] att[s, t*16+jb]  (unnormalized)
            xa_ps = psxa.tile([P, BJ], F32, tag="xa")
            for j in range(HPC):
                for b in range(B):
                    col = j * B + b
                    for t in range(NT):
                        nc.tensor.matmul(
                            xa_ps[:, col:col + 1],
                            xn[j][:, (b * NT + t) * P:(b * NT + t + 1) * P],
                            att[:, t * BJ + col:t * BJ + col + 1],
                            start=(t == 0), stop=(t == NT - 1),
                        )
            xa = spool.tile([P, BJ], BF16, tag="xa_sb")
            nc.vector.tensor_mul(xa[:, :], xa_ps[:, :], rec[:, :])

            # out[e, jb] = sum_d Wv_j[d, e] xa[d, jb] + cnt[e, jb]
            o_ps = pso.tile([P, BJ], F32, tag="o")
            for j in range(HPC):
                nc.tensor.matmul(o_ps[:, j * B:(j + 1) * B], wv[j],
                                 xa[:, j * B:(j + 1) * B], start=True, stop=True)
            fin = spool.tile([P, BJ], F32, tag="fin")
            nc.vector.tensor_add(fin[:, :], o_ps[:, :], cnt_f32[:, :])
            nc.sync.dma_start(out=out_t[:, :], in_=fin[:, :])
    nc.finalize()
    return nc


def _get_nc():
    if "nc" not in _NC_CACHE:
        _NC_CACHE["nc"] = _build_nc()
    return _NC_CACHE["nc"]


def _pos_bias_f32():
    """t5_position_bucket exactly as the reference computes it, sliced to the window."""
    if "pos" not in _NC_CACHE:
        import jax.numpy as jnp
        NUM_BUCKETS, MAX_DISTANCE = 32, 128
        n = (S - 1) - jnp.arange(S)
        max_exact = NUM_BUCKETS // 2
        is_small = n < max_exact
        large = max_exact + (
            jnp.log(jnp.maximum(n, 1).astype(jnp.float32) / max_exact)
            / np.log(MAX_DISTANCE / max_exact)
            * (NUM_BUCKETS - max_exact)
        ).astype(jnp.int32)
        large = jnp.minimum(large, NUM_BUCKETS - 1)
        pos = jnp.where(is_small, n, large).astype(jnp.float32)
        _NC_CACHE["pos"] = np.asarray(pos)[CUTOFF:]  # [W]
    return _NC_CACHE["pos"]


def kernel(**inputs) -> np.ndarray:
    import ml_dtypes
    BF = ml_dtypes.bfloat16
    F8N = ml_dtypes.float8_e4m3

    t = int(np.asarray(inputs["t"]))
    assert t == T, f"kernel hardcoded for t={T}, got {t}"
    content_t = np.asarray(inputs["content_t"], dtype=np.float32)
    cache = np.asarray(inputs["cache"], dtype=np.float32)
    Wq = np.asarray(inputs["Wq"], dtype=np.float32)
    Wk = np.asarray(inputs["Wk"], dtype=np.float32)
    Wv = np.asarray(inputs["Wv"], dtype=np.float32)
    pos_param = np.float32(np.asarray(inputs["pos_param"]))

    posb = (-pos_param * _pos_bias_f32()).astype(np.float32)       # [W]
    bias_col = np.ascontiguousarray(posb.reshape(NT, P).T)          # [p, t]

    # per-head folded score matrix MT_h = Wq_h @ Wk_h^T / sqrt(128)
    MT = (np.einsum("hde,hfe->hdf", Wq, Wk) * np.float32(KQS / np.sqrt(128.0)))
    MT_bf = MT.astype(BF)                                           # [H, d', d]
    Wv_bf = Wv.astype(BF)                                           # [H, d, e]

    cnt_h = content_t.reshape(B, H, P)                              # [B, H, 128]
    # full window incl. content row, cast to bf16 once: [B, W, H, P]
    w_all = np.empty((B, W, H, P), dtype=F8N)
    w_all[:, : W - 1] = cache[:, CUTOFF:T, :].reshape(B, W - 1, H, P).astype(F8N)
    w_all[:, W - 1] = cnt_h.astype(F8N)
    # [B, t, p, H, d] view for packing
    w_t = w_all.reshape(B, NT, P, H, P)

    ones8 = np.ones((P, P), dtype=F8N)

    in_maps = []
    for c in range(NCORES):
        h0 = HPC * c
        # xn[j][p, (b*NT+t)*P + d] = w[b, t*P+p, h0+j, d]
        blk = w_t[:, :, :, h0:h0 + HPC, :]                          # [B, t, p, j, d]
        xn_host = np.ascontiguousarray(
            blk.transpose(3, 2, 0, 1, 4).reshape(HPC, P, B * NT * P))
        # xt[j][d, (b*NT+t)*P + p] = w[b, t*P+p, h0+j, d]
        xt_host = np.ascontiguousarray(
            blk.transpose(3, 4, 0, 1, 2).reshape(HPC, P, B * NT * P))

        cnt_j = cnt_h[:, h0:h0 + HPC, :].transpose(2, 1, 0).reshape(P, BJ)  # [d, j*8+b]
        cb_host = np.ascontiguousarray(np.concatenate(
            [MT_bf[h0], MT_bf[h0 + 1], Wv_bf[h0], Wv_bf[h0 + 1],
             cnt_j.astype(BF)], axis=1))
        cf_host = np.ascontiguousarray(np.concatenate(
            [bias_col, cnt_j.astype(np.float32)], axis=1))
        in_maps.append({"xt": xt_host, "xn": xn_host, "cb": cb_host, "cf": cf_host, "on8": ones8})

    nc = _get_nc()
    res = run_bass_kernel_spmd(nc, in_maps, list(range(NCORES)), **_RUN_KWARGS)
    outs = np.stack([np.asarray(res.results[c]["out"]) for c in range(NCORES)])
    if not np.isfinite(outs).all():
        # The math here is provably finite (softmax denominator >= W*exp(-2));
        # a NaN/Inf can only be a transient device fault -- run once more.
        res = run_bass_kernel_spmd(nc, in_maps, list(range(NCORES)), **_RUN_KWARGS)
        outs = np.stack([np.asarray(res.results[c]["out"]) for c in range(NCORES)])
    _NC_CACHE["last_results"] = res
    # outs: [core, e, j*8+b] -> out_full[b, (2c+j)*128 + e]
    out_full = outs.reshape(NCORES, P, HPC, B).transpose(3, 0, 2, 1).reshape(B, H * P)
    return np.ascontiguousarray(out_full, dtype=np.float32)


_RUN_KWARGS = {}  # test harness may set {"trace": True, "tmpdir": ...}
